# revision 9
# baseline (speedup 1.0000x reference)
"""BERT self-attention (B=2, S=2048, H=1024, 16 heads) on 8 TRN2 NeuronCores.

Sharding: tensor-parallel over heads — 2 heads per core. Each core computes
Q/K/V projections for its head slice (contraction over the full hidden dim),
then attention for its (batch, head) pairs, producing the context transposed
[2*64, B*S]. The host concatenates the 8 per-core slices into [B, S, H].

Device-side layout choices (all matmuls fp16 — fp8 was measured to cost
3-6% output error through the softmax's concentrated rows, over the 2%
budget):
  - X is fed pre-transposed ([H, B*S]) so projections run with hidden on the
    partition (contraction) axis; Q^T and K^T come out in [d, token] layout,
    which is exactly what the scores matmul needs.
  - Scores are computed transposed (S^T = K Q^T) per 128-wide k-chunk, two
    heads packed into the PE array concurrently via row tiling (contraction
    is only d=64).
  - exp() runs on the scalar engine straight out of PSUM with the additive
    mask folded into the activation bias and 1/sqrt(d) into its scale.
  - V (+bias) sits in [k, d] fp16 layout with a ones column per head; the
    PV matmul accumulates context and the softmax denominator in one pass.
  - Normalization: native reciprocal on the [1, 512] denominator row (DVE),
    partition-broadcast on the idle GPSIMD engine, one multiply per head,
    fp16 output. No PE broadcast matmul, no PSUM->SBUF staging copy.
  - All DMA issues live on the sync/gpsimd queues so the scalar engine
    (the exp bottleneck) only runs activations.
"""

import sys
import types

sys.path.insert(0, "/opt/trn_rl_repo")

import numpy as np

# NTFF profiling hook (missing from this image's antenv): only needed when
# tracing; install if available, degrade silently otherwise.
try:
    import antenv.axon_hooks  # noqa: F401
except ImportError:
    try:
        from trn_agent_boot.trn_boot import _ntff_profile_via_ctypes

        _m = types.ModuleType("antenv.axon_hooks")
        _hook = _ntff_profile_via_ctypes("/opt/axon/libaxon_pjrt.so")
        _m.get_axon_ntff_profile_hook = lambda: _hook
        _m.set_axon_ntff_profile_hook = lambda h: None
        sys.modules["antenv.axon_hooks"] = _m
    except Exception:
        pass

import concourse.tile as tile
from concourse import bacc, mybir
from concourse.tile_rust import add_dep_helper
from concourse.bass_utils import run_bass_kernel_spmd

F32 = mybir.dt.float32
F16 = mybir.dt.float16
EXP = mybir.ActivationFunctionType.Exp

B, S, H, NHEADS, D = 2, 2048, 1024, 16, 64
T = B * S                # 4096 tokens
DPC = 128                # output dims per core (2 heads x 64)
NCORES = 8
NKC = S // 128           # 16 k-chunks per batch
NQB = S // 512           # 4 q-blocks of 512 per batch
NCI = H // 128           # 8 hidden (contraction) chunks

last_exec_time_ns = None
last_results = None

_cache = {}


def _build():
    nc = bacc.Bacc(
        "TRN2", target_bir_lowering=False, debug=False, enable_asserts=False
    )
    xt = nc.declare_dram_parameter("xt", [H, T], F16, isOutput=False)
    wq = nc.declare_dram_parameter("wq", [H, DPC], F16, isOutput=False)
    wk = nc.declare_dram_parameter("wk", [H, DPC], F16, isOutput=False)
    wv = nc.declare_dram_parameter("wv", [H, DPC], F16, isOutput=False)
    bq = nc.declare_dram_parameter("bq", [DPC, 1], F32, isOutput=False)
    bk = nc.declare_dram_parameter("bk", [DPC, 1], F32, isOutput=False)
    bvb = nc.declare_dram_parameter("bvb", [128, DPC], F32, isOutput=False)
    msk = nc.declare_dram_parameter("msk", [128, B * NKC], F32, isOutput=False)
    out = nc.declare_dram_parameter("out", [DPC, T], F16, isOutput=True)

    xt_r = xt.rearrange("(c p) t -> p c t", p=128)   # [128, 8, T]
    wq_r = wq.rearrange("(c p) d -> p c d", p=128)   # [128, 8, 128]
    wk_r = wk.rearrange("(c p) d -> p c d", p=128)
    wv_r = wv.rearrange("(c p) d -> p c d", p=128)

    with tile.TileContext(nc) as tc:
        with tc.tile_pool(name="persist", bufs=1) as pp:
            wq_sb = pp.tile([128, NCI, 128], F16, tag="wq")
            wk_sb = pp.tile([128, NCI, 128], F16, tag="wk")
            wv_sb = pp.tile([128, NCI, 128], F16, tag="wv")
            bq_sb = pp.tile([DPC, 1], F32, tag="bq")
            bk_sb = pp.tile([DPC, 1], F32, tag="bk")
            bvb_sb = pp.tile([128, 2, 64], F32, tag="bvb")
            msk_sb = pp.tile([128, B * NKC], F32, tag="msk")
            qt_sb = pp.tile([128, T], F16, tag="qt")
            kt_sb = pp.tile([128, T], F16, tag="kt")
            # V in [k, d] layout + a ones column per head:
            # [128, b, kc, head, 65] fp16
            vx = pp.tile([128, B, NKC, 2, 65], F16, tag="vx")

            nc.sync.dma_start(wk_sb[:], wk_r[:])
            nc.sync.dma_start(wq_sb[:], wq_r[:])
            nc.sync.dma_start(wv_sb[:], wv_r[:])
            nc.sync.dma_start(msk_sb[:], msk[:])
            nc.sync.dma_start(bq_sb[:], bq[:])
            nc.sync.dma_start(bk_sb[:], bk[:])
            nc.sync.dma_start(bvb_sb[:], bvb.rearrange("p (h d) -> p h d", h=2))
            for b_i in range(B):
                for kc in range(NKC):
                    nc.vector.memset(vx[:, b_i, kc, 0, 64:65], 1.0)
                    nc.vector.memset(vx[:, b_i, kc, 1, 64:65], 1.0)

            # ---- Phase 1: projections ----
            # Batch 0 first, K before Q/V, so batch-0 attention (which needs
            # all of K(b0) but only the first q-block of Q) can start early.
            xtp = tc.alloc_tile_pool(name="xtp", bufs=8)

            def dma_xt(tb, split=False):
                xt_t = xtp.tile([128, NCI, 512], F16, tag="xt", name="xt_t")
                if split:
                    # halve the first tile's load latency across two queues
                    nc.sync.dma_start(
                        xt_t[:, 0:4, :],
                        xt_r[:, 0:4, tb * 512:(tb + 1) * 512],
                    )
                    nc.gpsimd.dma_start(
                        xt_t[:, 4:8, :],
                        xt_r[:, 4:8, tb * 512:(tb + 1) * 512],
                    )
                else:
                    nc.gpsimd.dma_start(
                        xt_t[:], xt_r[:, :, tb * 512:(tb + 1) * 512]
                    )
                return xt_t

            b0_tiles = []
            b0_tiles.append(dma_xt(0, split=True))
            for tb in range(1, 4):
                b0_tiles.append(dma_xt(tb, split=(tb == 1)))

            # ---- Phase 2: attention (batch-1 projections woven in) ----
            with tc.tile_pool(name="stp", bufs=2, space="PSUM") as stp, \
                 tc.tile_pool(name="ctxq", bufs=2, space="PSUM") as ctxq, \
                 tc.tile_pool(name="trnp", bufs=2, space="PSUM") as trnp, \
                 tc.tile_pool(name="esp", bufs=3) as esp, \
                 tc.tile_pool(name="smallp", bufs=4) as smallp:
                qt_done = {}
                kt_done = {}
                vx_done = {}

                def proj_qk2(kind, tb, xt_t):
                    w_sb, t_sb, b_sb, done = (
                        (wq_sb, qt_sb, bq_sb, qt_done) if kind == "q"
                        else (wk_sb, kt_sb, bk_sb, kt_done)
                    )
                    ps = trnp.tile([128, 512], F32, tag="trans", name="pj_ps0")
                    for ci in range(NCI):
                        nc.tensor.matmul(
                            ps[:], w_sb[:, ci, :], xt_t[:, ci, :],
                            start=(ci == 0), stop=(ci == NCI - 1),
                        )
                    col = tb * 512
                    done[tb] = nc.vector.tensor_scalar_add(
                        t_sb[:, col:col + 512], ps[:], b_sb[:, 0:1]
                    )

                def emit_v_tt(tb, tt):
                    v_ps = trnp.tile([128, 2, 64], F32, tag="trans",
                                     name="v_ps")
                    for ci in range(NCI):
                        nc.tensor.matmul(
                            v_ps[:],
                            xt_t_of[tb][:, ci, tt * 128:(tt + 1) * 128],
                            wv_sb[:, ci, :],
                            start=(ci == 0), stop=(ci == NCI - 1),
                        )
                    g = tb * 4 + tt
                    b_i, kc = g // NKC, g % NKC
                    vx_done[(b_i, kc)] = nc.vector.tensor_add(
                        vx[:, b_i, kc, :, 0:64], v_ps[:], bvb_sb[:]
                    )

                def normalize(ctx0, ctx1, qcol):
                    cout = smallp.tile([128, 512], F16, tag="cout",
                                       name="cout")
                    for h, ctx in ((0, ctx0), (1, ctx1)):
                        rb = smallp.tile([1, 512], F32, tag=f"rb{h}",
                                         name=f"rb{h}")
                        nc.vector.reciprocal(rb[:], ctx[64:65, :])
                        rbb = smallp.tile([64, 512], F32, tag=f"rbb{h}",
                                          name=f"rbb{h}")
                        nc.gpsimd.partition_broadcast(rbb[:], rb[:])
                        nc.vector.tensor_mul(
                            cout[h * 64:(h + 1) * 64, :],
                            ctx[0:64, :], rbb[:],
                        )
                    nc.sync.dma_start(out[0:64, qcol:qcol + 512],
                                      cout[0:64, :])
                    nc.gpsimd.dma_start(out[64:128, qcol:qcol + 512],
                                        cout[64:128, :])

                # Work queue of small projection chunks, drained a few
                # matmuls at a time between score chunks so the scalar
                # engine (the bottleneck) is never starved.
                work = []
                xt_t_of = {}

                def wq_proj(kind, tb, xt_t):
                    # split one projection into 4 chunks of 2 ci-steps
                    state = {}

                    def chunk(ci0, kind=kind, tb=tb, xt_t=xt_t, state=state):
                        w_sb, t_sb, b_sb = (
                            (wq_sb, qt_sb, bq_sb) if kind == "q"
                            else (wk_sb, kt_sb, bk_sb)
                        )
                        if ci0 == 0:
                            state["ps"] = trnp.tile(
                                [128, 512], F32, tag="trans", name="pj_ps"
                            )
                        ps_t = state["ps"]
                        for ci in (ci0, ci0 + 1):
                            nc.tensor.matmul(
                                ps_t[:], w_sb[:, ci, :], xt_t[:, ci, :],
                                start=(ci == 0), stop=(ci == NCI - 1),
                            )
                        if ci0 == NCI - 2:
                            col = tb * 512
                            done = (qt_done if kind == "q" else kt_done)
                            done[tb] = nc.vector.tensor_scalar_add(
                                t_sb[:, col:col + 512], ps_t[:], b_sb[:, 0:1]
                            )
                    for ci0 in range(0, NCI, 2):
                        work.append(lambda c=ci0: chunk(c))

                def wq_vproj(tb):
                    for tt in range(4):
                        work.append(lambda t=tt, b=tb: emit_v_tt(b, t))

                def filler(b_i, qb):
                    if b_i == 0 and qb == 0:
                        for tt in (2, 3):
                            work.append(lambda t=tt: emit_v_tt(0, t))
                        for tb in (1, 2, 3):
                            wq_proj("k", tb, b0_tiles[tb])
                            wq_vproj(tb)
                        for tb in (1, 2, 3):
                            wq_proj("q", tb, b0_tiles[tb])
                    elif b_i == 0 and qb == 1:
                        for tb in range(4, 8):
                            t = dma_xt(tb)
                            b1_tiles.append(t)
                            xt_t_of[tb] = t
                        for tb in (4, 5, 6, 7):
                            wq_proj("k", tb, b1_tiles[tb - 4])
                    elif b_i == 0 and qb == 2:
                        for tb in (4, 5, 6, 7):
                            wq_vproj(tb)
                        wq_proj("q", 4, b1_tiles[0])
                    elif b_i == 0 and qb == 3:
                        for tb in (5, 6, 7):
                            wq_proj("q", tb, b1_tiles[tb - 4])

                # batch-0 head-start projections
                for tb in range(4):
                    xt_t_of[tb] = b0_tiles[tb]
                proj_qk2("k", 0, b0_tiles[0])
                proj_qk2("q", 0, b0_tiles[0])
                emit_v_tt(0, 0)
                emit_v_tt(0, 1)

                b1_tiles = []
                for b_i in range(B):
                    for qb in range(NQB):
                        filler(b_i, qb)
                        qcol = b_i * S + qb * 512
                        ctx0 = ctxq.tile([65, 512], F32, tag="cx")
                        ctx1 = ctxq.tile([65, 512], F32, tag="cx")
                        for kc in range(NKC):
                            for _ in range(2):
                                if work:
                                    work.pop(0)()
                            ktb = b_i * 4 + kc // 4
                            qtb = b_i * 4 + qb
                            while work and not (
                                ktb in kt_done and qtb in qt_done
                                and (b_i, kc) in vx_done
                            ):
                                work.pop(0)()
                            kcol = b_i * S + kc * 128
                            st = stp.tile([128, 1024], F32, tag="st")
                            m0 = nc.tensor.matmul(
                                st[:, 0:512],
                                kt_sb[0:64, kcol:kcol + 128],
                                qt_sb[0:64, qcol:qcol + 512],
                                start=True, stop=True, tile_position=(0, 0),
                            )
                            m1 = nc.tensor.matmul(
                                st[:, 512:1024],
                                kt_sb[64:128, kcol:kcol + 128],
                                qt_sb[64:128, qcol:qcol + 512],
                                start=True, stop=True, tile_position=(64, 0),
                            )
                            for m in (m0, m1):
                                add_dep_helper(m.ins, kt_done[ktb].ins,
                                               True, "kt ready")
                                add_dep_helper(m.ins, qt_done[qtb].ins,
                                               True, "qt ready")
                            est = esp.tile([128, 1024], F16, tag="est",
                                           name="est")
                            nc.scalar.activation(
                                est[:], st[:], EXP, scale=0.125,
                                bias=msk_sb[:, b_i * NKC + kc:
                                            b_i * NKC + kc + 1],
                            )
                            p0 = nc.tensor.matmul(
                                ctx0[:], vx[:, b_i, kc, 0, :],
                                est[:, 0:512],
                                start=(kc == 0), stop=(kc == NKC - 1),
                            )
                            p1 = nc.tensor.matmul(
                                ctx1[:], vx[:, b_i, kc, 1, :],
                                est[:, 512:1024],
                                start=(kc == 0), stop=(kc == NKC - 1),
                            )
                            vd = vx_done[(b_i, kc)]
                            add_dep_helper(p0.ins, vd.ins, True, "vx")
                            add_dep_helper(p1.ins, vd.ins, True, "vx")
                        normalize(ctx0, ctx1, qcol)
                while work:
                    work.pop(0)()
            xtp.release()

    nc.compile()
    return nc


def kernel(hidden_states, attention_mask, Wq, bq, Wk, bk, Wv, bv, trace=False):
    global last_exec_time_ns, last_results
    x = np.asarray(hidden_states, dtype=np.float32)
    mask = np.asarray(attention_mask, dtype=np.float32)
    Wq = np.asarray(Wq, dtype=np.float32)
    Wk = np.asarray(Wk, dtype=np.float32)
    Wv = np.asarray(Wv, dtype=np.float32)
    bq = np.asarray(bq, dtype=np.float32)
    bk = np.asarray(bk, dtype=np.float32)
    bv = np.asarray(bv, dtype=np.float32)

    if "nc" not in _cache:
        _cache["nc"] = _build()
    nc = _cache["nc"]

    xt = np.ascontiguousarray(x.reshape(T, H).T).astype(np.float16)  # [H, T]
    # mask columns: [p, b*16+kc] = mask[b, kc*128+p]
    mcols = np.ascontiguousarray(
        mask.reshape(B, NKC, 128).transpose(2, 0, 1).reshape(128, B * NKC)
    )
    in_maps = []
    for c in range(NCORES):
        sl = slice(c * DPC, (c + 1) * DPC)
        in_maps.append({
            "xt": xt,
            "wq": np.ascontiguousarray(Wq[:, sl]).astype(np.float16),
            "wk": np.ascontiguousarray(Wk[:, sl]).astype(np.float16),
            "wv": np.ascontiguousarray(Wv[:, sl]).astype(np.float16),
            "bq": np.ascontiguousarray(bq[sl, None]),
            "bk": np.ascontiguousarray(bk[sl, None]),
            "bvb": np.ascontiguousarray(
                np.broadcast_to(bv[sl][None, :], (128, DPC))
            ),
            "msk": mcols,
        })

    res = run_bass_kernel_spmd(
        nc, in_maps, core_ids=list(range(NCORES)), trace=trace
    )
    last_exec_time_ns = res.exec_time_ns
    last_results = res

    # assemble: per-core out [128, T] -> [B, S, 128]; concat over cores
    parts = [
        res.results[c]["out"].astype(np.float32).reshape(DPC, B, S)
        .transpose(1, 2, 0)
        for c in range(NCORES)
    ]
    return np.ascontiguousarray(np.concatenate(parts, axis=2))


# revision 13
# speedup vs baseline: 1.1956x; 1.1956x over previous
"""BERT self-attention (B=2, S=2048, H=1024, 16 heads) on 8 TRN2 NeuronCores.

Sharding: tensor-parallel over heads — 2 heads per core. Each core computes
Q/K/V projections for its head slice (contraction over the full hidden dim),
then attention for its (batch, head) pairs, producing the context transposed
[2*64, B*S]. The host concatenates the 8 per-core slices into [B, S, H].

Device-side layout choices (all matmuls fp16 — fp8 was measured to cost
3-6% output error through the softmax's concentrated rows, over the 2%
budget):
  - X is fed pre-transposed ([H, B*S]) so projections run with hidden on the
    partition (contraction) axis; Q^T and K^T come out in [d, token] layout,
    which is exactly what the scores matmul needs.
  - Scores are computed transposed (S^T = K Q^T) per 128-wide k-chunk, two
    heads packed into the PE array concurrently via row tiling (contraction
    is only d=64).
  - exp() runs on the scalar engine straight out of PSUM with the additive
    mask folded into the activation bias and 1/sqrt(d) into its scale.
  - V (+bias) sits in [k, d] fp16 layout with a ones column per head; the
    PV matmul accumulates context and the softmax denominator in one pass.
  - Normalization: native reciprocal on the [1, 512] denominator row (DVE),
    partition-broadcast on the idle GPSIMD engine, one multiply per head,
    fp16 output. No PE broadcast matmul, no PSUM->SBUF staging copy.
  - All DMA issues live on the sync/gpsimd queues so the scalar engine
    (the exp bottleneck) only runs activations.
"""

import sys
import types

sys.path.insert(0, "/opt/trn_rl_repo")

import numpy as np

# NTFF profiling hook (missing from this image's antenv): only needed when
# tracing; install if available, degrade silently otherwise.
try:
    import antenv.axon_hooks  # noqa: F401
except ImportError:
    try:
        from trn_agent_boot.trn_boot import _ntff_profile_via_ctypes

        _m = types.ModuleType("antenv.axon_hooks")
        _hook = _ntff_profile_via_ctypes("/opt/axon/libaxon_pjrt.so")
        _m.get_axon_ntff_profile_hook = lambda: _hook
        _m.set_axon_ntff_profile_hook = lambda h: None
        sys.modules["antenv.axon_hooks"] = _m
    except Exception:
        pass

import concourse.tile as tile
from concourse import bacc, mybir
from concourse.tile_rust import add_dep_helper
from concourse.bass_utils import run_bass_kernel_spmd

F32 = mybir.dt.float32
F16 = mybir.dt.float16
EXP = mybir.ActivationFunctionType.Exp

B, S, H, NHEADS, D = 2, 2048, 1024, 16, 64
T = B * S                # 4096 tokens
DPC = 128                # output dims per core (2 heads x 64)
NCORES = 8
NKC = S // 128           # 16 k-chunks per batch
NQB = S // 512           # 4 q-blocks of 512 per batch
NCI = H // 128           # 8 hidden (contraction) chunks

last_exec_time_ns = None
last_results = None

_cache = {}


def _build():
    nc = bacc.Bacc(
        "TRN2", target_bir_lowering=False, debug=False, enable_asserts=False
    )
    xt = nc.declare_dram_parameter("xt", [H, T], F16, isOutput=False)
    wq = nc.declare_dram_parameter("wq", [H, DPC], F16, isOutput=False)
    wk = nc.declare_dram_parameter("wk", [H, DPC], F16, isOutput=False)
    wv = nc.declare_dram_parameter("wv", [H, DPC], F16, isOutput=False)
    bq = nc.declare_dram_parameter("bq", [DPC, 1], F32, isOutput=False)
    bk = nc.declare_dram_parameter("bk", [DPC, 1], F32, isOutput=False)
    bvb = nc.declare_dram_parameter("bvb", [128, DPC], F32, isOutput=False)
    msk = nc.declare_dram_parameter("msk", [128, B * NKC], F32, isOutput=False)
    out = nc.declare_dram_parameter("out", [DPC, T], F16, isOutput=True)

    xt_r = xt.rearrange("(c p) t -> p c t", p=128)   # [128, 8, T]
    wq_r = wq.rearrange("(c p) d -> p c d", p=128)   # [128, 8, 128]
    wk_r = wk.rearrange("(c p) d -> p c d", p=128)
    wv_r = wv.rearrange("(c p) d -> p c d", p=128)

    with tile.TileContext(nc) as tc:
        with tc.tile_pool(name="persist", bufs=1) as pp:
            wq_sb = pp.tile([128, NCI, 128], F16, tag="wq")
            wk_sb = pp.tile([128, NCI, 128], F16, tag="wk")
            wv_sb = pp.tile([128, NCI, 128], F16, tag="wv")
            bq_sb = pp.tile([DPC, 1], F32, tag="bq")
            bk_sb = pp.tile([DPC, 1], F32, tag="bk")
            bvb_sb = pp.tile([128, 2, 64], F32, tag="bvb")
            msk_sb = pp.tile([128, B * NKC], F32, tag="msk")
            qt_sb = pp.tile([128, T], F16, tag="qt")
            kt_sb = pp.tile([128, T], F16, tag="kt")
            # V in [k, d] layout, two ones columns first (ctx rows 0/1 get
            # the softmax denominator at base partition 0 for the custom-DVE
            # reciprocal), zero pad to col 32 so the context block starts at
            # partition 64 (64-partition PSUM reads must be 64-aligned):
            # [128, b, kc, head, 128] fp16
            vx = pp.tile([128, B, NKC, 2, 128], F16, tag="vx")

            nc.sync.dma_start(wk_sb[:], wk_r[:])
            nc.sync.dma_start(wq_sb[:], wq_r[:])
            nc.sync.dma_start(wv_sb[:], wv_r[:])
            nc.sync.dma_start(msk_sb[:], msk[:])
            nc.sync.dma_start(bq_sb[:], bq[:])
            nc.sync.dma_start(bk_sb[:], bk[:])
            nc.sync.dma_start(bvb_sb[:], bvb.rearrange("p (h d) -> p h d", h=2))
            nc.vector.memset(vx[:], 0.0)
            for b_i in range(B):
                for kc in range(NKC):
                    nc.vector.memset(vx[:, b_i, kc, 0, 0:2], 1.0)
                    nc.vector.memset(vx[:, b_i, kc, 1, 0:2], 1.0)

            # ---- Phase 1: projections ----
            # Batch 0 first, K before Q/V, so batch-0 attention (which needs
            # all of K(b0) but only the first q-block of Q) can start early.
            xtp = tc.alloc_tile_pool(name="xtp", bufs=8)

            def dma_xt(tb, split=False):
                xt_t = xtp.tile([128, NCI, 512], F16, tag="xt", name="xt_t")
                if split:
                    # halve the first tile's load latency across two queues
                    nc.sync.dma_start(
                        xt_t[:, 0:4, :],
                        xt_r[:, 0:4, tb * 512:(tb + 1) * 512],
                    )
                    nc.gpsimd.dma_start(
                        xt_t[:, 4:8, :],
                        xt_r[:, 4:8, tb * 512:(tb + 1) * 512],
                    )
                else:
                    nc.gpsimd.dma_start(
                        xt_t[:], xt_r[:, :, tb * 512:(tb + 1) * 512]
                    )
                return xt_t

            b0_tiles = []
            b0_tiles.append(dma_xt(0, split=True))
            for tb in range(1, 4):
                b0_tiles.append(dma_xt(tb, split=(tb == 1)))

            # ---- Phase 2: attention (batch-1 projections woven in) ----
            with tc.tile_pool(name="stp", bufs=2, space="PSUM") as stp, \
                 tc.tile_pool(name="ctxq", bufs=2, space="PSUM") as ctxq, \
                 tc.tile_pool(name="trnp", bufs=2, space="PSUM") as trnp, \
                 tc.tile_pool(name="esp", bufs=3) as esp, \
                 tc.tile_pool(name="smallp", bufs=4) as smallp:
                qt_done = {}
                kt_done = {}
                vx_done = {}

                def proj_qk2(kind, tb, xt_t):
                    w_sb, t_sb, b_sb, done = (
                        (wq_sb, qt_sb, bq_sb, qt_done) if kind == "q"
                        else (wk_sb, kt_sb, bk_sb, kt_done)
                    )
                    ps = trnp.tile([128, 512], F32, tag="trans", name="pj_ps0")
                    for ci in range(NCI):
                        nc.tensor.matmul(
                            ps[:], w_sb[:, ci, :], xt_t[:, ci, :],
                            start=(ci == 0), stop=(ci == NCI - 1),
                        )
                    col = tb * 512
                    done[tb] = nc.vector.tensor_scalar_add(
                        t_sb[:, col:col + 512], ps[:], b_sb[:, 0:1]
                    )

                def emit_v_tt(tb, tt):
                    v_ps = trnp.tile([128, 2, 64], F32, tag="trans",
                                     name="v_ps")
                    for ci in range(NCI):
                        nc.tensor.matmul(
                            v_ps[:],
                            xt_t_of[tb][:, ci, tt * 128:(tt + 1) * 128],
                            wv_sb[:, ci, :],
                            start=(ci == 0), stop=(ci == NCI - 1),
                        )
                    g = tb * 4 + tt
                    b_i, kc = g // NKC, g % NKC
                    vx_done[(b_i, kc)] = nc.vector.tensor_add(
                        vx[:, b_i, kc, :, 64:128], v_ps[:], bvb_sb[:]
                    )

                def normalize(ctx0, ctx1, qcol):
                    cout = smallp.tile([128, 512], F16, tag="cout",
                                       name="cout")
                    for h, ctx in ((0, ctx0), (1, ctx1)):
                        rb = smallp.tile([2, 512], F32, tag=f"rb{h}",
                                         name=f"rb{h}")
                        nc.vector.reciprocal_approx_fast(rb[:],
                                                         ctx[0:2, :])
                        rbb = smallp.tile([64, 512], F32, tag=f"rbb{h}",
                                          name=f"rbb{h}")
                        nc.gpsimd.partition_broadcast(rbb[:], rb[0:1, :])
                        nc.vector.tensor_mul(
                            cout[h * 64:(h + 1) * 64, :],
                            ctx[64:128, :], rbb[:],
                        )
                    nc.sync.dma_start(out[0:64, qcol:qcol + 512],
                                      cout[0:64, :])
                    nc.gpsimd.dma_start(out[64:128, qcol:qcol + 512],
                                        cout[64:128, :])

                # Work queue of small projection chunks, drained a few
                # matmuls at a time between score chunks so the scalar
                # engine (the bottleneck) is never starved.
                work = []
                xt_t_of = {}

                def wq_proj(kind, tb, xt_t):
                    # split one projection into 4 chunks of 2 ci-steps
                    state = {}

                    def chunk(ci0, kind=kind, tb=tb, xt_t=xt_t, state=state):
                        w_sb, t_sb, b_sb = (
                            (wq_sb, qt_sb, bq_sb) if kind == "q"
                            else (wk_sb, kt_sb, bk_sb)
                        )
                        if ci0 == 0:
                            state["ps"] = trnp.tile(
                                [128, 512], F32, tag="trans", name="pj_ps"
                            )
                        ps_t = state["ps"]
                        for ci in (ci0, ci0 + 1):
                            nc.tensor.matmul(
                                ps_t[:], w_sb[:, ci, :], xt_t[:, ci, :],
                                start=(ci == 0), stop=(ci == NCI - 1),
                            )
                        if ci0 == NCI - 2:
                            col = tb * 512
                            done = (qt_done if kind == "q" else kt_done)
                            done[tb] = nc.vector.tensor_scalar_add(
                                t_sb[:, col:col + 512], ps_t[:], b_sb[:, 0:1]
                            )
                    for ci0 in range(0, NCI, 2):
                        work.append(lambda c=ci0: chunk(c))

                def wq_vproj(tb):
                    for tt in range(4):
                        work.append(lambda t=tt, b=tb: emit_v_tt(b, t))

                def filler(b_i, qb):
                    if b_i == 0 and qb == 0:
                        for tt in (2, 3):
                            work.append(lambda t=tt: emit_v_tt(0, t))
                        for tb in (1, 2, 3):
                            wq_proj("k", tb, b0_tiles[tb])
                            wq_vproj(tb)
                        for tb in (1, 2, 3):
                            wq_proj("q", tb, b0_tiles[tb])
                    elif b_i == 0 and qb == 1:
                        for tb in range(4, 8):
                            t = dma_xt(tb)
                            b1_tiles.append(t)
                            xt_t_of[tb] = t
                        for tb in (4, 5, 6, 7):
                            wq_proj("k", tb, b1_tiles[tb - 4])
                    elif b_i == 0 and qb == 2:
                        for tb in (4, 5, 6, 7):
                            wq_vproj(tb)
                        wq_proj("q", 4, b1_tiles[0])
                    elif b_i == 0 and qb == 3:
                        for tb in (5, 6, 7):
                            wq_proj("q", tb, b1_tiles[tb - 4])

                # batch-0 head-start projections
                for tb in range(4):
                    xt_t_of[tb] = b0_tiles[tb]
                proj_qk2("k", 0, b0_tiles[0])
                proj_qk2("q", 0, b0_tiles[0])
                emit_v_tt(0, 0)
                emit_v_tt(0, 1)

                b1_tiles = []
                for b_i in range(B):
                    for qb in range(NQB):
                        filler(b_i, qb)
                        qcol = b_i * S + qb * 512
                        ctx0 = ctxq.tile([128, 512], F32, tag="cx")
                        ctx1 = ctxq.tile([128, 512], F32, tag="cx")
                        for kc in range(NKC):
                            for _ in range(2):
                                if work:
                                    work.pop(0)()
                            ktb = b_i * 4 + kc // 4
                            qtb = b_i * 4 + qb
                            while work and not (
                                ktb in kt_done and qtb in qt_done
                                and (b_i, kc) in vx_done
                            ):
                                work.pop(0)()
                            kcol = b_i * S + kc * 128
                            st = stp.tile([128, 1024], F32, tag="st")
                            m0 = nc.tensor.matmul(
                                st[:, 0:512],
                                kt_sb[0:64, kcol:kcol + 128],
                                qt_sb[0:64, qcol:qcol + 512],
                                start=True, stop=True, tile_position=(0, 0),
                            )
                            m1 = nc.tensor.matmul(
                                st[:, 512:1024],
                                kt_sb[64:128, kcol:kcol + 128],
                                qt_sb[64:128, qcol:qcol + 512],
                                start=True, stop=True, tile_position=(64, 0),
                            )
                            for m in (m0, m1):
                                add_dep_helper(m.ins, kt_done[ktb].ins,
                                               True, "kt ready")
                                add_dep_helper(m.ins, qt_done[qtb].ins,
                                               True, "qt ready")
                            est = esp.tile([128, 1024], F16, tag="est",
                                           name="est")
                            nc.scalar.activation(
                                est[:], st[:], EXP, scale=0.125,
                                bias=msk_sb[:, b_i * NKC + kc:
                                            b_i * NKC + kc + 1],
                            )
                            p0 = nc.tensor.matmul(
                                ctx0[:], vx[:, b_i, kc, 0, :],
                                est[:, 0:512],
                                start=(kc == 0), stop=(kc == NKC - 1),
                            )
                            p1 = nc.tensor.matmul(
                                ctx1[:], vx[:, b_i, kc, 1, :],
                                est[:, 512:1024],
                                start=(kc == 0), stop=(kc == NKC - 1),
                            )
                            vd = vx_done[(b_i, kc)]
                            add_dep_helper(p0.ins, vd.ins, True, "vx")
                            add_dep_helper(p1.ins, vd.ins, True, "vx")
                        normalize(ctx0, ctx1, qcol)
                while work:
                    work.pop(0)()
            xtp.release()

    nc.compile()
    return nc


def kernel(hidden_states, attention_mask, Wq, bq, Wk, bk, Wv, bv, trace=False):
    global last_exec_time_ns, last_results
    x = np.asarray(hidden_states, dtype=np.float32)
    mask = np.asarray(attention_mask, dtype=np.float32)
    Wq = np.asarray(Wq, dtype=np.float32)
    Wk = np.asarray(Wk, dtype=np.float32)
    Wv = np.asarray(Wv, dtype=np.float32)
    bq = np.asarray(bq, dtype=np.float32)
    bk = np.asarray(bk, dtype=np.float32)
    bv = np.asarray(bv, dtype=np.float32)

    if "nc" not in _cache:
        _cache["nc"] = _build()
    nc = _cache["nc"]

    xt = np.ascontiguousarray(x.reshape(T, H).T).astype(np.float16)  # [H, T]
    # mask columns: [p, b*16+kc] = mask[b, kc*128+p]
    mcols = np.ascontiguousarray(
        mask.reshape(B, NKC, 128).transpose(2, 0, 1).reshape(128, B * NKC)
    )
    in_maps = []
    for c in range(NCORES):
        sl = slice(c * DPC, (c + 1) * DPC)
        in_maps.append({
            "xt": xt,
            "wq": np.ascontiguousarray(Wq[:, sl]).astype(np.float16),
            "wk": np.ascontiguousarray(Wk[:, sl]).astype(np.float16),
            "wv": np.ascontiguousarray(Wv[:, sl]).astype(np.float16),
            "bq": np.ascontiguousarray(bq[sl, None]),
            "bk": np.ascontiguousarray(bk[sl, None]),
            "bvb": np.ascontiguousarray(
                np.broadcast_to(bv[sl][None, :], (128, DPC))
            ),
            "msk": mcols,
        })

    res = run_bass_kernel_spmd(
        nc, in_maps, core_ids=list(range(NCORES)), trace=trace
    )
    last_exec_time_ns = res.exec_time_ns
    last_results = res

    # assemble: per-core out [128, T] -> [B, S, 128]; concat over cores
    parts = [
        res.results[c]["out"].astype(np.float32).reshape(DPC, B, S)
        .transpose(1, 2, 0)
        for c in range(NCORES)
    ]
    return np.ascontiguousarray(np.concatenate(parts, axis=2))
